# revision 20
# baseline (speedup 1.0000x reference)
"""Trainium2 Bass kernel for a 4-layer LSTM-style stack with local+global logits.

Computation (per example row x of the [16384, 512] input):
    h0 = 0, c0 = 0
    for i in 1..4:
        z  = [x, h_{i-1}] @ W{f,i,o,c} + b        (4 gates, K = 1024)
        c  = tanh(z_c) * sigmoid(z_i) + sigmoid(z_f) * c
        h  = sigmoid(z_o) * tanh(c)
        local_i = h @ Wl_i + bl_i
    global = [x, h4] @ Wg + bg
Returns (concat(local_1..4) [16384, 960], global [16384, 960]).

Strategy (v2):
  - Data-parallel over 8 cores: 2048 rows each, weights replicated.
  - Z = x @ W_top + b computed once per example (bf16), reused by all 4
    layers; layer 1 needs no matmul at all.
  - The recurrent matmuls h @ W_bot (3 layers) run in fp8 e4m3 with
    perf_mode=DoubleRow: contraction 256 per pass, 2 passes, ~2x PE rate.
    Z / locals / global stay bf16 (fp8 there fails the accuracy gate).
  - Z is accumulated into the gate PSUM with an identity-weight matmul, so
    the ACT engine reads complete pre-activations straight from PSUM and
    the DVE never touches the (slow, fp32) PSUM pre-add path.
  - Elementwise work is done on [128, 2048] tiles (4 hid-tiles wide),
    split per half for pipelining; Z bias-copies and the h->fp8 casts run
    on the otherwise-idle GPSIMD engine.
"""

import os
import sys

import numpy as np

for _p in ("/opt/trn_rl_repo", "/root/.axon_site/_ro/trn_rl_repo"):
    if os.path.isdir(_p) and _p not in sys.path:
        sys.path.insert(0, _p)

import ml_dtypes

import concourse.bass as bass
import concourse.tile as tile
from concourse import bacc, mybir
from concourse.bass_utils import run_bass_kernel_spmd

BF16 = mybir.dt.bfloat16
F32 = mybir.dt.float32
FP8 = mybir.dt.float8e4
AF = mybir.ActivationFunctionType
ALU = mybir.AluOpType
DR = mybir.MatmulPerfMode.DoubleRow

N_CORES = 8
N = 16384
K = 512                  # input features
U = 512                  # hidden units
GF = 4 * U               # 2048 concatenated gate features (order f, i, o, c)
MC = N // N_CORES        # 2048 rows per core
NQ = 4                   # quarters per core
EXQ = MC // NQ           # 512 examples per quarter
NCLS = [64, 128, 256, 512]
OFFS = [0, 64, 192, 448]
BL4OFF = [0, 256, 768, 1792]   # offsets into the 4x-tiled local-bias tile
TOT = 960

# per-gate source of the Z term in layers 2..4: 'id' accumulates Z into
# PSUM via an identity matmul (PE), 'dve' adds Z to PSUM on the DVE.
GATE_MODES = ("id", "id", "id", "id")

LAST_RESULT = None       # BassKernelResults of the most recent run (for test.py)


def _build_program(gate_modes=GATE_MODES):
    nc = bacc.Bacc("TRN2", target_bir_lowering=False, debug=False)

    xt_d = nc.dram_tensor("xt", [K, MC], BF16, kind="ExternalInput")
    wtop_d = nc.dram_tensor("wtop", [K, GF], BF16, kind="ExternalInput")
    w80_d = nc.dram_tensor("w80", [128, 2, GF], FP8, kind="ExternalInput")
    w81_d = nc.dram_tensor("w81", [128, 2, GF], FP8, kind="ExternalInput")
    wl_d = nc.dram_tensor("wl", [U, TOT], BF16, kind="ExternalInput")
    wg_d = nc.dram_tensor("wg", [K + U, TOT], BF16, kind="ExternalInput")
    ident_d = nc.dram_tensor("ident", [128, 128], BF16, kind="ExternalInput")
    bgate_d = nc.dram_tensor("bgate", [128, 16], F32, kind="ExternalInput")
    bl4_d = nc.dram_tensor("bl4", [128, 3840], BF16, kind="ExternalInput")
    bgrep_d = nc.dram_tensor("bgrep", [128, TOT], BF16, kind="ExternalInput")
    # bf16 outputs: f32 DVE writes run at half rate and double the DMA bytes
    oloc_d = nc.dram_tensor("oloc", [MC, TOT], BF16, kind="ExternalOutput")
    oglb_d = nc.dram_tensor("oglb", [MC, TOT], BF16, kind="ExternalOutput")

    with tile.TileContext(nc) as tc:
        with (
            tc.tile_pool(name="wpool", bufs=1) as wpool,
            tc.tile_pool(name="xpool", bufs=4) as xpool,
            tc.tile_pool(name="zpool", bufs=2) as zpool,
            tc.tile_pool(name="gpool", bufs=2) as gpool,
            tc.tile_pool(name="cpool", bufs=2) as cpool,
            tc.tile_pool(name="hpool", bufs=4) as hpool,
            tc.tile_pool(name="h8pool", bufs=3) as h8pool,
            tc.tile_pool(name="ttp", bufs=2) as ttp,
            tc.tile_pool(name="tcp", bufs=2) as tcp,
            tc.tile_pool(name="prep", bufs=2) as prep,
            tc.tile_pool(name="lop", bufs=2) as lop,
            tc.tile_pool(name="glop", bufs=3) as glop,
            tc.tile_pool(name="gpsum", bufs=2, space="PSUM") as gpsum,
        ):
            # ---- resident weights/biases --------------------------------
            # First Z matmul needs only x(q0) + the g=0 column group of
            # W_top, so those bytes are DMAed first.
            wtop_sb = [[None] * 4 for _ in range(4)]   # [kt][g]
            xs = {}

            def dma_wtop(g):
                for kt in range(4):
                    t = wpool.tile([128, 512], BF16, tag=f"wt{kt}g{g}")
                    nc.sync.dma_start(
                        t[:], wtop_d[kt * 128:(kt + 1) * 128,
                                     g * 512:(g + 1) * 512])
                    wtop_sb[kt][g] = t

            def dma_x(q):
                xs[q] = []
                for kt in range(4):
                    t = xpool.tile([128, EXQ], BF16, tag=f"x{kt}")
                    nc.sync.dma_start(
                        t[:], xt_d[kt * 128:(kt + 1) * 128,
                                   q * EXQ:(q + 1) * EXQ])
                    xs[q].append(t)

            dma_wtop(0)
            dma_x(0)
            bgate_sb = wpool.tile([128, 16], F32, tag="bgate")
            nc.sync.dma_start(bgate_sb[:], bgate_d[:])
            dma_wtop(1)
            dma_x(1)
            dma_wtop(2)
            dma_wtop(3)
            w8_sb = []
            for j, d in enumerate((w80_d, w81_d)):
                t = wpool.tile([128, 2, GF], FP8, tag=f"w8{j}")
                nc.sync.dma_start(t[:], d[:])
                w8_sb.append(t)
            id_sb = wpool.tile([128, 128], BF16, tag="ident")
            nc.sync.dma_start(id_sb[:], ident_d[:])
            wl_sb = []
            for kt in range(4):
                t = wpool.tile([128, TOT], BF16, tag=f"wl{kt}")
                nc.sync.dma_start(t[:], wl_d[kt * 128:(kt + 1) * 128, :])
                wl_sb.append(t)
            bl4_sb = wpool.tile([128, 3840], BF16, tag="bl4")
            nc.sync.dma_start(bl4_sb[:], bl4_d[:])
            wg_sb = []
            for kt in range(8):
                t = wpool.tile([128, TOT], BF16, tag=f"wg{kt}")
                nc.sync.dma_start(t[:], wg_d[kt * 128:(kt + 1) * 128, :])
                wg_sb.append(t)
            bgrep_sb = wpool.tile([128, TOT], BF16, tag="bgrep")
            nc.sync.dma_start(bgrep_sb[:], bgrep_d[:])

            zs = {}      # (q) -> [4 Z tiles, [128, 2048] bf16, gate-major]
            cs = {}      # (q) -> c tile [128, 2048] bf16
            hs = {}      # (q, layer) -> h tile [128, 2048] bf16
            h8s = {}     # (q, layer) -> h8 tile [128, 4, 512] fp8

            def z_phase(q):
                """Z_g = x @ Wtop_g + b_g for the 4 gates (bf16, in SBUF)."""
                if q in zs:
                    return
                if q not in xs:
                    dma_x(q)
                zs[q] = []
                for g in range(4):
                    zt = zpool.tile([128, GF], BF16, tag=f"z{g}")
                    ps = gpsum.tile([128, 2048], F32, tag="ps")
                    for t in range(4):
                        sl = slice(t * 512, (t + 1) * 512)
                        for kt in range(4):
                            nc.tensor.matmul(
                                ps[:, sl],
                                wtop_sb[kt][g][:, t * 128:(t + 1) * 128],
                                xs[q][kt][:],
                                start=(kt == 0), stop=(kt == 3))
                    for t in range(4):
                        of = g * 4 + t
                        sl = slice(t * 512, (t + 1) * 512)
                        # GPSIMD cannot read PSUM -> this stays on the DVE
                        nc.vector.tensor_scalar(
                            zt[:, sl], ps[:, sl],
                            bgate_sb[:, of:of + 1], None, ALU.add)
                    zs[q].append(zt)

            def cand(q, layer, G):
                """c = G_i*G_c (+ G_f*c); h = G_o * tanh(c); h8 = fp8(h)."""
                ht = hpool.tile([128, GF], BF16, tag="h")
                h8t = None
                if layer < 4:   # layer 4's h feeds no further recurrence
                    h8t = h8pool.tile([128, 4, 512], FP8, tag="h8", name="h8t")
                for j in range(2):
                    sl = slice(j * 1024, (j + 1) * 1024)
                    if layer == 1:
                        nc.vector.tensor_mul(
                            cs[q][:, sl], G[1][:, sl], G[3][:, sl])
                    else:
                        t1 = ttp.tile([128, 1024], BF16, tag="t1")
                        nc.vector.tensor_mul(t1[:], G[1][:, sl], G[3][:, sl])
                        t2 = ttp.tile([128, 1024], BF16, tag="t2")
                        # f-gate is the first activation done -> this mul is
                        # off the critical path and GPSIMD is otherwise idle
                        nc.gpsimd.tensor_mul(t2[:], G[0][:, sl], cs[q][:, sl])
                        nc.vector.tensor_add(cs[q][:, sl], t1[:], t2[:])
                    tc_t = tcp.tile([128, 1024], BF16, tag="tc")
                    nc.scalar.activation(tc_t[:], cs[q][:, sl], AF.Tanh)
                    if h8t is not None:
                        # fused second mul straight to fp8: one dep hop
                        # shorter than mul+cast on the h8->matmul chain
                        nc.vector.tensor_mul(h8t[:, 2 * j:2 * j + 2, :],
                                             G[2][:, sl], tc_t[:])
                    nc.vector.tensor_mul(ht[:, sl], G[2][:, sl], tc_t[:])
                hs[(q, layer)] = ht
                if h8t is not None:
                    h8s[(q, layer)] = h8t

            def l1(q):
                """Layer 1: h0 = 0, so gates come straight from Z."""
                cs[q] = cpool.tile([128, GF], BF16, tag="c", name="c")
                G = [None] * 4
                for g in (1, 2, 3):
                    gt = gpool.tile([128, GF], BF16, tag=f"G{g}")
                    func = AF.Tanh if g == 3 else AF.Sigmoid
                    for jj in range(2):
                        jsl = slice(jj * 1024, (jj + 1) * 1024)
                        nc.scalar.activation(gt[:, jsl], zs[q][g][:, jsl],
                                             func)
                    G[g] = gt
                cand(q, 1, G)

            def rec(q, layer):
                """Layers 2..4: z = Z + h_prev @ W_bot (fp8 DoubleRow)."""
                h8p = h8s[(q, layer - 1)]
                G = []
                for g in range(4):
                    ps = gpsum.tile([128, 2048], F32, tag="ps")
                    use_id = gate_modes[g] == "id"
                    for t in range(4):
                        sl = slice(t * 512, (t + 1) * 512)
                        col = (g * 4 + t) * 128
                        if use_id:
                            nc.tensor.matmul(
                                ps[:, sl], id_sb[:], zs[q][g][:, sl],
                                start=True, stop=False,
                                skip_group_check=True)
                        nc.tensor.matmul(
                            ps[:, sl], w8_sb[0][:, :, col:col + 128],
                            h8p[:, 0:2, :],
                            start=(not use_id), stop=False,
                            perf_mode=DR, skip_group_check=True)
                        nc.tensor.matmul(
                            ps[:, sl], w8_sb[1][:, :, col:col + 128],
                            h8p[:, 2:4, :],
                            start=False, stop=True,
                            perf_mode=DR, skip_group_check=True)
                    gt = gpool.tile([128, GF], BF16, tag=f"G{g}")
                    func = AF.Tanh if g == 3 else AF.Sigmoid
                    if use_id:
                        # per-half acts: byte-range deps let the first half
                        # start while the second half's matmuls still run
                        for jj in range(2):
                            jsl = slice(jj * 1024, (jj + 1) * 1024)
                            nc.scalar.activation(gt[:, jsl], ps[:, jsl], func)
                    else:
                        pre = prep.tile([128, GF], BF16, tag=f"pre{g}")
                        nc.vector.tensor_tensor(
                            pre[:], ps[:], zs[q][g][:], ALU.add)
                        nc.scalar.activation(gt[:], pre[:], func)
                    G.append(gt)
                cand(q, layer, G)

            def loc(q, layer):
                """local_{layer} = h_{layer} @ Wl + bl, natural layout."""
                li = layer - 1
                off, ncl = OFFS[li], NCLS[li]
                ht = hs[(q, layer)]
                ps = gpsum.tile([128, 2048], F32, tag="ps")
                for e in range(4):
                    osl = slice(e * ncl, (e + 1) * ncl)
                    for t in range(4):
                        nc.tensor.matmul(
                            ps[:, osl],
                            ht[:, t * 512 + e * 128:t * 512 + e * 128 + 128],
                            wl_sb[t][:, off:off + ncl],
                            start=(t == 0 and (e * ncl) % 512 == 0),
                            stop=(t == 3 and e == 3),
                            skip_group_check=True)
                w4 = 4 * ncl
                st = lop.tile([128, 2048], BF16, tag="lo")
                nc.vector.tensor_tensor(
                    st[:, 0:w4], ps[:, 0:w4],
                    bl4_sb[:, BL4OFF[li]:BL4OFF[li] + w4], ALU.add)
                for e in range(4):
                    r0 = q * EXQ + e * 128
                    nc.sync.dma_start(
                        oloc_d[r0:r0 + 128, off:off + ncl],
                        st[:, e * ncl:(e + 1) * ncl])

            def gl_ep(q, ep):
                """global = [x, h4] @ Wg + bg for one pair of e-tiles."""
                h4 = hs[(q, 4)]
                ps = gpsum.tile([128, 2048], F32, tag="ps", name="ps")
                for ei in range(2):
                    e = ep * 2 + ei
                    for s0, s1 in ((0, 512), (512, TOT)):
                        osl = slice(ei * 1024 + s0, ei * 1024 + s1)
                        for kt in range(8):
                            if kt < 4:
                                lh = xs[q][kt][:, e * 128:(e + 1) * 128]
                            else:
                                t = kt - 4
                                lh = h4[:, t * 512 + e * 128:
                                        t * 512 + e * 128 + 128]
                            nc.tensor.matmul(
                                ps[:, osl], lh, wg_sb[kt][:, s0:s1],
                                start=(kt == 0), stop=(kt == 7),
                                skip_group_check=True)
                for ei in range(2):
                    e = ep * 2 + ei
                    st = glop.tile([128, TOT], BF16, tag="glo", name="glo")
                    nc.vector.tensor_tensor(
                        st[:], ps[:, ei * 1024:ei * 1024 + TOT],
                        bgrep_sb[:], ALU.add)
                    r0 = q * EXQ + e * 128
                    nc.sync.dma_start(oglb_d[r0:r0 + 128, :], st[:])

            # ---- schedule ----------------------------------------------
            # GL(1) chunks are deferred into pair-2's layer steps: they are
            # the only spare PE work able to cover those chain tails.
            for a, b in ((0, 1), (2, 3)):
                z_phase(a)
                z_phase(b)
                l1(a)
                l1(b)
                for layer in (2, 3, 4):
                    rec(a, layer)
                    rec(b, layer)
                    loc(a, layer - 1)
                    loc(b, layer - 1)
                    if a == 2 and layer < 4:
                        gl_ep(1, layer - 2)
                if b == 1:
                    z_phase(2)
                    loc(a, 4)
                    gl_ep(0, 0)
                    z_phase(3)
                    loc(b, 4)
                    gl_ep(0, 1)
                else:
                    loc(a, 4)
                    gl_ep(2, 0)
                    gl_ep(2, 1)
                    loc(b, 4)
                    gl_ep(3, 0)
                    gl_ep(3, 1)

    nc.compile()
    return nc


_PROGRAM = None


def _get_program():
    global _PROGRAM
    if _PROGRAM is None:
        _PROGRAM = _build_program()
    return _PROGRAM


def _prep_weights(Wf, Wi, Wo, Wc, bf, bi, bo, bc,
                  Wl0, bl0, Wl1, bl1, Wl2, bl2, Wl3, bl3, Wg, bg):
    bf16 = ml_dtypes.bfloat16
    e4m3 = ml_dtypes.float8_e4m3

    wcat = np.concatenate(
        [np.asarray(w, np.float32) for w in (Wf, Wi, Wo, Wc)], axis=1)
    wtop = np.ascontiguousarray(wcat[:K]).astype(bf16)          # [512, 2048]
    wbot = wcat[K:]                                             # [512, 2048]
    wb = wbot.reshape(2, 2, 128, GF)                            # [j, i, p, m]
    w80 = np.ascontiguousarray(wb[0].transpose(1, 0, 2)).astype(e4m3)
    w81 = np.ascontiguousarray(wb[1].transpose(1, 0, 2)).astype(e4m3)

    bcat = np.concatenate(
        [np.asarray(x, np.float32) for x in (bf, bi, bo, bc)])  # [2048]
    bgate = np.ascontiguousarray(bcat.reshape(16, 128).T)       # [128, 16]

    wl = np.concatenate(
        [np.asarray(w, np.float32) for w in (Wl0, Wl1, Wl2, Wl3)],
        axis=1).astype(bf16)                                    # [512, 960]
    blcat = np.concatenate(
        [np.asarray(x, np.float32) for x in (bl0, bl1, bl2, bl3)])
    bl4 = np.concatenate(
        [np.tile(blcat[OFFS[i]:OFFS[i] + NCLS[i]], 4) for i in range(4)])
    bl4 = np.ascontiguousarray(
        np.broadcast_to(bl4, (128, 3840))).astype(bf16)
    wg = np.asarray(Wg, np.float32).astype(bf16)                # [1024, 960]
    bgrep = np.ascontiguousarray(
        np.broadcast_to(np.asarray(bg, np.float32), (128, TOT))).astype(bf16)
    ident = np.eye(128, dtype=np.float32).astype(bf16)

    return {
        "wtop": wtop, "w80": w80, "w81": w81, "wl": wl, "wg": wg,
        "ident": ident, "bgate": bgate, "bl4": bl4, "bgrep": bgrep,
    }


def kernel(inputs, Wf, bf, Wi, bi, Wo, bo, Wc, bc,
           Wl0, bl0, Wl1, bl1, Wl2, bl2, Wl3, bl3, Wg, bg):
    global LAST_RESULT
    bf16 = ml_dtypes.bfloat16

    inputs = np.ascontiguousarray(np.asarray(inputs, dtype=np.float32))
    xt_all = inputs.T.astype(bf16)                    # [512, 16384]
    shared = _prep_weights(Wf, Wi, Wo, Wc, bf, bi, bo, bc,
                           Wl0, bl0, Wl1, bl1, Wl2, bl2, Wl3, bl3, Wg, bg)

    in_maps = []
    for c in range(N_CORES):
        m = {"xt": np.ascontiguousarray(xt_all[:, c * MC:(c + 1) * MC])}
        m.update(shared)
        in_maps.append(m)

    nc = _get_program()
    trace = os.environ.get("BASS_KERNEL_TRACE", "0") == "1"
    tmpdir = os.environ.get("BASS_KERNEL_TMPDIR") or None
    res = run_bass_kernel_spmd(
        nc, in_maps, list(range(N_CORES)), trace=trace, tmpdir=tmpdir)
    LAST_RESULT = res

    loc = np.concatenate(
        [np.asarray(r["oloc"], np.float32) for r in res.results], axis=0)
    glb = np.concatenate(
        [np.asarray(r["oglb"], np.float32) for r in res.results], axis=0)
    return loc, glb


# revision 27
# speedup vs baseline: 1.3119x; 1.3119x over previous
"""Trainium2 Bass kernel for a 4-layer LSTM-style stack with local+global logits.

Computation (per example row x of the [16384, 512] input):
    h0 = 0, c0 = 0
    for i in 1..4:
        z  = [x, h_{i-1}] @ W{f,i,o,c} + b        (4 gates, K = 1024)
        c  = tanh(z_c) * sigmoid(z_i) + sigmoid(z_f) * c
        h  = sigmoid(z_o) * tanh(c)
        local_i = h @ Wl_i + bl_i
    global = [x, h4] @ Wg + bg
Returns (concat(local_1..4) [16384, 960], global [16384, 960]).

Strategy (v2):
  - Data-parallel over 8 cores: 2048 rows each, weights replicated.
  - Z = x @ W_top + b computed once per example (bf16), reused by all 4
    layers; layer 1 needs no matmul at all.
  - The recurrent matmuls h @ W_bot (3 layers) run in fp8 e4m3 with
    perf_mode=DoubleRow: contraction 256 per pass, 2 passes, ~2x PE rate.
    Z / locals / global stay bf16 (fp8 there fails the accuracy gate).
  - Z is accumulated into the gate PSUM with an identity-weight matmul, so
    the ACT engine reads complete pre-activations straight from PSUM and
    the DVE never touches the (slow, fp32) PSUM pre-add path.
  - Elementwise work is done on [128, 2048] tiles (4 hid-tiles wide),
    split per half for pipelining; Z bias-copies and the h->fp8 casts run
    on the otherwise-idle GPSIMD engine.
"""

import os
import sys

import numpy as np

for _p in ("/opt/trn_rl_repo", "/root/.axon_site/_ro/trn_rl_repo"):
    if os.path.isdir(_p) and _p not in sys.path:
        sys.path.insert(0, _p)

import ml_dtypes

import concourse.bass as bass
import concourse.tile as tile
from concourse import bacc, mybir
from concourse.bass_utils import run_bass_kernel_spmd

BF16 = mybir.dt.bfloat16
F32 = mybir.dt.float32
FP8 = mybir.dt.float8e4
AF = mybir.ActivationFunctionType
ALU = mybir.AluOpType
DR = mybir.MatmulPerfMode.DoubleRow

N_CORES = 8
N = 16384
K = 512                  # input features
U = 512                  # hidden units
GF = 4 * U               # 2048 concatenated gate features (order f, i, o, c)
MC = N // N_CORES        # 2048 rows per core
NQ = 4                   # quarters per core
EXQ = MC // NQ           # 512 examples per quarter
NCLS = [64, 128, 256, 512]
OFFS = [0, 64, 192, 448]
BL4OFF = [0, 256, 768, 1792]   # offsets into the 4x-tiled local-bias tile
TOT = 960

# per-gate source of the Z term in layers 2..4: 'id' accumulates Z into
# PSUM via an identity matmul (PE), 'dve' adds Z to PSUM on the DVE.
GATE_MODES = ("id", "id", "id", "id")

LAST_RESULT = None       # BassKernelResults of the most recent run (for test.py)


def _build_program(gate_modes=GATE_MODES):
    nc = bacc.Bacc("TRN2", target_bir_lowering=False, debug=False)

    xt_d = nc.dram_tensor("xt", [K, MC], BF16, kind="ExternalInput")
    wtop_d = nc.dram_tensor("wtop", [K, GF], BF16, kind="ExternalInput")
    w80_d = nc.dram_tensor("w80", [128, 2, GF], FP8, kind="ExternalInput")
    w81_d = nc.dram_tensor("w81", [128, 2, GF], FP8, kind="ExternalInput")
    wl_d = nc.dram_tensor("wl", [U, TOT], BF16, kind="ExternalInput")
    wg_d = nc.dram_tensor("wg", [K + U, TOT], BF16, kind="ExternalInput")
    ident_d = nc.dram_tensor("ident", [128, 128], BF16, kind="ExternalInput")
    bgate_d = nc.dram_tensor("bgate", [128, 16], F32, kind="ExternalInput")
    bl4_d = nc.dram_tensor("bl4", [128, 3840], BF16, kind="ExternalInput")
    bgrep_d = nc.dram_tensor("bgrep", [128, TOT], BF16, kind="ExternalInput")
    # bf16 outputs: f32 DVE writes run at half rate and double the DMA bytes
    oloc_d = nc.dram_tensor("oloc", [MC, TOT], BF16, kind="ExternalOutput")
    oglb_d = nc.dram_tensor("oglb", [MC, TOT], BF16, kind="ExternalOutput")

    with tile.TileContext(nc) as tc:
        with (
            tc.tile_pool(name="wpool", bufs=1) as wpool,
            tc.tile_pool(name="xpool", bufs=4) as xpool,
            tc.tile_pool(name="zpool", bufs=2) as zpool,
            tc.tile_pool(name="gpool", bufs=2) as gpool,
            tc.tile_pool(name="cpool", bufs=2) as cpool,
            tc.tile_pool(name="hpool", bufs=4) as hpool,
            tc.tile_pool(name="h8pool", bufs=3) as h8pool,
            tc.tile_pool(name="ttp", bufs=2) as ttp,
            tc.tile_pool(name="tcp", bufs=2) as tcp,
            tc.tile_pool(name="prep", bufs=2) as prep,
            tc.tile_pool(name="lop", bufs=2) as lop,
            tc.tile_pool(name="glop", bufs=3) as glop,
            tc.tile_pool(name="gpsum", bufs=4, space="PSUM") as gpsum,
        ):
            # ---- resident weights/biases --------------------------------
            # First Z matmul needs only x(q0) + the g=0 column group of
            # W_top, so those bytes are DMAed first.
            wtop_sb = [[None] * 4 for _ in range(4)]   # [kt][g]
            xs = {}

            def dma_wtop(g):
                for kt in range(4):
                    t = wpool.tile([128, 512], BF16, tag=f"wt{kt}g{g}")
                    nc.sync.dma_start(
                        t[:], wtop_d[kt * 128:(kt + 1) * 128,
                                     g * 512:(g + 1) * 512])
                    wtop_sb[kt][g] = t

            def dma_x(q):
                xs[q] = []
                for kt in range(4):
                    t = xpool.tile([128, EXQ], BF16, tag=f"x{kt}")
                    nc.sync.dma_start(
                        t[:], xt_d[kt * 128:(kt + 1) * 128,
                                   q * EXQ:(q + 1) * EXQ])
                    xs[q].append(t)

            dma_wtop(0)
            dma_x(0)
            bgate_sb = wpool.tile([128, 16], F32, tag="bgate")
            nc.sync.dma_start(bgate_sb[:], bgate_d[:])
            dma_wtop(1)
            dma_x(1)
            dma_wtop(2)
            dma_wtop(3)
            w8_sb = []
            for j, d in enumerate((w80_d, w81_d)):
                t = wpool.tile([128, 2, GF], FP8, tag=f"w8{j}")
                nc.sync.dma_start(t[:], d[:])
                w8_sb.append(t)
            id_sb = wpool.tile([128, 128], BF16, tag="ident")
            nc.sync.dma_start(id_sb[:], ident_d[:])
            wl_sb = []
            for kt in range(4):
                t = wpool.tile([128, TOT], BF16, tag=f"wl{kt}")
                nc.sync.dma_start(t[:], wl_d[kt * 128:(kt + 1) * 128, :])
                wl_sb.append(t)
            bl4_sb = wpool.tile([128, 3840], BF16, tag="bl4")
            nc.sync.dma_start(bl4_sb[:], bl4_d[:])
            wg_sb = []
            for kt in range(8):
                t = wpool.tile([128, TOT], BF16, tag=f"wg{kt}")
                nc.sync.dma_start(t[:], wg_d[kt * 128:(kt + 1) * 128, :])
                wg_sb.append(t)
            bgrep_sb = wpool.tile([128, TOT], BF16, tag="bgrep")
            nc.sync.dma_start(bgrep_sb[:], bgrep_d[:])

            zs = {}      # (q) -> [4 Z tiles, [128, 2048] bf16, gate-major]
            cs = {}      # (q) -> c tile [128, 2048] bf16
            hs = {}      # (q, layer) -> h tile [128, 2048] bf16
            h8s = {}     # (q, layer) -> h8 tile [128, 4, 512] fp8

            def z_phase(q):
                """Z_g = x @ Wtop_g + b_g for the 4 gates (bf16, in SBUF)."""
                if q in zs:
                    return
                if q not in xs:
                    dma_x(q)
                zs[q] = []
                for g in range(4):
                    zt = zpool.tile([128, GF], BF16, tag=f"z{g}")
                    for half in range(2):
                        ps = gpsum.tile([128, 1024], F32, tag="ps",
                                        name="ps")
                        for ti in range(2):
                            t = half * 2 + ti
                            sl = slice(ti * 512, (ti + 1) * 512)
                            for kt in range(4):
                                nc.tensor.matmul(
                                    ps[:, sl],
                                    wtop_sb[kt][g][:, t * 128:(t + 1) * 128],
                                    xs[q][kt][:],
                                    start=(kt == 0), stop=(kt == 3))
                        for ti in range(2):
                            t = half * 2 + ti
                            of = g * 4 + t
                            sl = slice(ti * 512, (ti + 1) * 512)
                            zsl = slice(t * 512, (t + 1) * 512)
                            # GPSIMD cannot read PSUM -> stays on the DVE
                            nc.vector.tensor_scalar(
                                zt[:, zsl], ps[:, sl],
                                bgate_sb[:, of:of + 1], None, ALU.add)
                    zs[q].append(zt)

            def cand(q, layer, G):
                """c = G_i*G_c (+ G_f*c); h = G_o * tanh(c); h8 = fp8(h)."""
                ht = hpool.tile([128, GF], BF16, tag="h")
                h8t = None
                if layer < 4:   # layer 4's h feeds no further recurrence
                    h8t = h8pool.tile([128, 4, 512], FP8, tag="h8", name="h8t")
                for j in range(2):
                    sl = slice(j * 1024, (j + 1) * 1024)
                    if layer == 1:
                        nc.vector.tensor_mul(
                            cs[q][:, sl], G[1][:, sl], G[3][:, sl])
                    else:
                        t1 = ttp.tile([128, 1024], BF16, tag="t1")
                        nc.vector.tensor_mul(t1[:], G[1][:, sl], G[3][:, sl])
                        t2 = ttp.tile([128, 1024], BF16, tag="t2")
                        # f-gate is the first activation done -> this mul is
                        # off the critical path and GPSIMD is otherwise idle
                        nc.gpsimd.tensor_mul(t2[:], G[0][:, sl], cs[q][:, sl])
                        nc.vector.tensor_add(cs[q][:, sl], t1[:], t2[:])
                    tc_t = tcp.tile([128, 1024], BF16, tag="tc")
                    nc.scalar.activation(tc_t[:], cs[q][:, sl], AF.Tanh)
                    nc.vector.tensor_mul(ht[:, sl], G[2][:, sl], tc_t[:])
                    if h8t is not None:
                        # DVE cast (~0.7us) beats the GPSIMD CAST (~3.6us)
                        # which sat on the h8 -> next-layer-matmul chain
                        nc.vector.tensor_copy(h8t[:, 2 * j:2 * j + 2, :],
                                              ht[:, sl])
                hs[(q, layer)] = ht
                if h8t is not None:
                    h8s[(q, layer)] = h8t

            def l1(q):
                """Layer 1: h0 = 0, so gates come straight from Z."""
                cs[q] = cpool.tile([128, GF], BF16, tag="c", name="c")
                G = [None] * 4
                for g in (1, 2, 3):
                    gt = gpool.tile([128, GF], BF16, tag=f"G{g}")
                    func = AF.Tanh if g == 3 else AF.Sigmoid
                    for jj in range(2):
                        jsl = slice(jj * 1024, (jj + 1) * 1024)
                        nc.scalar.activation(gt[:, jsl], zs[q][g][:, jsl],
                                             func)
                    G[g] = gt
                cand(q, 1, G)

            def rec(q, layer):
                """Layers 2..4: z = Z + h_prev @ W_bot (fp8 DoubleRow)."""
                h8p = h8s[(q, layer - 1)]
                G = []
                for g in range(4):
                    gt = gpool.tile([128, GF], BF16, tag=f"G{g}")
                    func = AF.Tanh if g == 3 else AF.Sigmoid
                    use_id = gate_modes[g] == "id"
                    for half in range(2):
                        ps = gpsum.tile([128, 1024], F32, tag="ps",
                                        name="ps")
                        for ti in range(2):
                            t = half * 2 + ti
                            sl = slice(ti * 512, (ti + 1) * 512)
                            zsl = slice(t * 512, (t + 1) * 512)
                            col = (g * 4 + t) * 128
                            if use_id:
                                nc.tensor.matmul(
                                    ps[:, sl], id_sb[:], zs[q][g][:, zsl],
                                    start=True, stop=False,
                                    skip_group_check=True)
                            nc.tensor.matmul(
                                ps[:, sl], w8_sb[0][:, :, col:col + 128],
                                h8p[:, 0:2, :],
                                start=(not use_id), stop=False,
                                perf_mode=DR, skip_group_check=True)
                            nc.tensor.matmul(
                                ps[:, sl], w8_sb[1][:, :, col:col + 128],
                                h8p[:, 2:4, :],
                                start=False, stop=True,
                                perf_mode=DR, skip_group_check=True)
                        jsl = slice(half * 1024, (half + 1) * 1024)
                        if use_id:
                            nc.scalar.activation(gt[:, jsl], ps[:], func)
                        else:
                            pre = prep.tile([128, GF], BF16, tag=f"pre{g}")
                            nc.vector.tensor_tensor(
                                pre[:, jsl], ps[:], zs[q][g][:, jsl],
                                ALU.add)
                            nc.scalar.activation(gt[:, jsl], pre[:, jsl],
                                                 func)
                    G.append(gt)
                cand(q, layer, G)

            def loc(q, layer):
                """local_{layer} = h_{layer} @ Wl + bl, natural layout."""
                li = layer - 1
                off, ncl = OFFS[li], NCLS[li]
                ht = hs[(q, layer)]
                st = lop.tile([128, 2048], BF16, tag="lo")
                for half in range(2):
                    ps = gpsum.tile([128, 1024], F32, tag="ps", name="ps")
                    for ei in range(2):
                        e = half * 2 + ei
                        osl = slice(ei * ncl, (ei + 1) * ncl)
                        for t in range(4):
                            nc.tensor.matmul(
                                ps[:, osl],
                                ht[:, t * 512 + e * 128:
                                    t * 512 + e * 128 + 128],
                                wl_sb[t][:, off:off + ncl],
                                start=(t == 0 and (ei * ncl) % 512 == 0),
                                stop=(t == 3 and ei == 1),
                                skip_group_check=True)
                    w2 = 2 * ncl
                    b0 = BL4OFF[li] + half * w2
                    nc.vector.tensor_tensor(
                        st[:, half * w2:half * w2 + w2], ps[:, 0:w2],
                        bl4_sb[:, b0:b0 + w2], ALU.add)
                for e in range(4):
                    r0 = q * EXQ + e * 128
                    nc.sync.dma_start(
                        oloc_d[r0:r0 + 128, off:off + ncl],
                        st[:, e * ncl:(e + 1) * ncl])

            def gl_ep(q, ep):
                """global = [x, h4] @ Wg + bg for one pair of e-tiles."""
                h4 = hs[(q, 4)]
                for ei in range(2):
                    e = ep * 2 + ei
                    ps = gpsum.tile([128, 1024], F32, tag="ps", name="ps")
                    for s0, s1 in ((0, 512), (512, TOT)):
                        for kt in range(8):
                            if kt < 4:
                                lh = xs[q][kt][:, e * 128:(e + 1) * 128]
                            else:
                                t = kt - 4
                                lh = h4[:, t * 512 + e * 128:
                                        t * 512 + e * 128 + 128]
                            nc.tensor.matmul(
                                ps[:, s0:s1], lh, wg_sb[kt][:, s0:s1],
                                start=(kt == 0), stop=(kt == 7),
                                skip_group_check=True)
                    st = glop.tile([128, TOT], BF16, tag="glo", name="glo")
                    nc.vector.tensor_tensor(
                        st[:], ps[:, 0:TOT], bgrep_sb[:], ALU.add)
                    r0 = q * EXQ + e * 128
                    nc.sync.dma_start(oglb_d[r0:r0 + 128, :], st[:])

            # ---- schedule ----------------------------------------------
            for a, b in ((0, 1), (2, 3)):
                z_phase(a)
                z_phase(b)
                l1(a)
                l1(b)
                for layer in (2, 3, 4):
                    rec(a, layer)
                    rec(b, layer)
                    loc(a, layer - 1)
                    loc(b, layer - 1)
                if b == 1:
                    z_phase(2)
                    loc(a, 4)
                    gl_ep(a, 0)
                    gl_ep(a, 1)
                    z_phase(3)
                    loc(b, 4)
                    gl_ep(b, 0)
                    gl_ep(b, 1)
                else:
                    loc(a, 4)
                    gl_ep(a, 0)
                    gl_ep(a, 1)
                    loc(b, 4)
                    gl_ep(b, 0)
                    gl_ep(b, 1)

    nc.compile()
    return nc


_PROGRAM = None


def _get_program():
    global _PROGRAM
    if _PROGRAM is None:
        _PROGRAM = _build_program()
    return _PROGRAM


def _prep_weights(Wf, Wi, Wo, Wc, bf, bi, bo, bc,
                  Wl0, bl0, Wl1, bl1, Wl2, bl2, Wl3, bl3, Wg, bg):
    bf16 = ml_dtypes.bfloat16
    e4m3 = ml_dtypes.float8_e4m3

    wcat = np.concatenate(
        [np.asarray(w, np.float32) for w in (Wf, Wi, Wo, Wc)], axis=1)
    wtop = np.ascontiguousarray(wcat[:K]).astype(bf16)          # [512, 2048]
    wbot = wcat[K:]                                             # [512, 2048]
    wb = wbot.reshape(2, 2, 128, GF)                            # [j, i, p, m]
    w80 = np.ascontiguousarray(wb[0].transpose(1, 0, 2)).astype(e4m3)
    w81 = np.ascontiguousarray(wb[1].transpose(1, 0, 2)).astype(e4m3)

    bcat = np.concatenate(
        [np.asarray(x, np.float32) for x in (bf, bi, bo, bc)])  # [2048]
    bgate = np.ascontiguousarray(bcat.reshape(16, 128).T)       # [128, 16]

    wl = np.concatenate(
        [np.asarray(w, np.float32) for w in (Wl0, Wl1, Wl2, Wl3)],
        axis=1).astype(bf16)                                    # [512, 960]
    blcat = np.concatenate(
        [np.asarray(x, np.float32) for x in (bl0, bl1, bl2, bl3)])
    bl4 = np.concatenate(
        [np.tile(blcat[OFFS[i]:OFFS[i] + NCLS[i]], 4) for i in range(4)])
    bl4 = np.ascontiguousarray(
        np.broadcast_to(bl4, (128, 3840))).astype(bf16)
    wg = np.asarray(Wg, np.float32).astype(bf16)                # [1024, 960]
    bgrep = np.ascontiguousarray(
        np.broadcast_to(np.asarray(bg, np.float32), (128, TOT))).astype(bf16)
    ident = np.eye(128, dtype=np.float32).astype(bf16)

    return {
        "wtop": wtop, "w80": w80, "w81": w81, "wl": wl, "wg": wg,
        "ident": ident, "bgate": bgate, "bl4": bl4, "bgrep": bgrep,
    }


def kernel(inputs, Wf, bf, Wi, bi, Wo, bo, Wc, bc,
           Wl0, bl0, Wl1, bl1, Wl2, bl2, Wl3, bl3, Wg, bg):
    global LAST_RESULT
    bf16 = ml_dtypes.bfloat16

    inputs = np.ascontiguousarray(np.asarray(inputs, dtype=np.float32))
    xt_all = inputs.T.astype(bf16)                    # [512, 16384]
    shared = _prep_weights(Wf, Wi, Wo, Wc, bf, bi, bo, bc,
                           Wl0, bl0, Wl1, bl1, Wl2, bl2, Wl3, bl3, Wg, bg)

    in_maps = []
    for c in range(N_CORES):
        m = {"xt": np.ascontiguousarray(xt_all[:, c * MC:(c + 1) * MC])}
        m.update(shared)
        in_maps.append(m)

    nc = _get_program()
    trace = os.environ.get("BASS_KERNEL_TRACE", "0") == "1"
    tmpdir = os.environ.get("BASS_KERNEL_TMPDIR") or None
    res = run_bass_kernel_spmd(
        nc, in_maps, list(range(N_CORES)), trace=trace, tmpdir=tmpdir)
    LAST_RESULT = res

    loc = np.concatenate(
        [np.asarray(r["oloc"], np.float32) for r in res.results], axis=0)
    glb = np.concatenate(
        [np.asarray(r["oglb"], np.float32) for r in res.results], axis=0)
    return loc, glb


# revision 29
# speedup vs baseline: 1.3264x; 1.0110x over previous
"""Trainium2 Bass kernel for a 4-layer LSTM-style stack with local+global logits.

Computation (per example row x of the [16384, 512] input):
    h0 = 0, c0 = 0
    for i in 1..4:
        z  = [x, h_{i-1}] @ W{f,i,o,c} + b        (4 gates, K = 1024)
        c  = tanh(z_c) * sigmoid(z_i) + sigmoid(z_f) * c
        h  = sigmoid(z_o) * tanh(c)
        local_i = h @ Wl_i + bl_i
    global = [x, h4] @ Wg + bg
Returns (concat(local_1..4) [16384, 960], global [16384, 960]).

Strategy (v2):
  - Data-parallel over 8 cores: 2048 rows each, weights replicated.
  - Z = x @ W_top + b computed once per example (bf16), reused by all 4
    layers; layer 1 needs no matmul at all.
  - The recurrent matmuls h @ W_bot (3 layers) run in fp8 e4m3 with
    perf_mode=DoubleRow: contraction 256 per pass, 2 passes, ~2x PE rate.
    Z / locals / global stay bf16 (fp8 there fails the accuracy gate).
  - Z is accumulated into the gate PSUM with an identity-weight matmul, so
    the ACT engine reads complete pre-activations straight from PSUM and
    the DVE never touches the (slow, fp32) PSUM pre-add path.
  - Elementwise work is done on [128, 2048] tiles (4 hid-tiles wide),
    split per half for pipelining; Z bias-copies and the h->fp8 casts run
    on the otherwise-idle GPSIMD engine.
"""

import os
import sys

import numpy as np

for _p in ("/opt/trn_rl_repo", "/root/.axon_site/_ro/trn_rl_repo"):
    if os.path.isdir(_p) and _p not in sys.path:
        sys.path.insert(0, _p)

import ml_dtypes

import concourse.bass as bass
import concourse.tile as tile
from concourse import bacc, mybir
from concourse.bass_utils import run_bass_kernel_spmd

BF16 = mybir.dt.bfloat16
F32 = mybir.dt.float32
FP8 = mybir.dt.float8e4
AF = mybir.ActivationFunctionType
ALU = mybir.AluOpType
DR = mybir.MatmulPerfMode.DoubleRow

N_CORES = 8
N = 16384
K = 512                  # input features
U = 512                  # hidden units
GF = 4 * U               # 2048 concatenated gate features (order f, i, o, c)
MC = N // N_CORES        # 2048 rows per core
NQ = 4                   # quarters per core
EXQ = MC // NQ           # 512 examples per quarter
NCLS = [64, 128, 256, 512]
OFFS = [0, 64, 192, 448]
BL4OFF = [0, 256, 768, 1792]   # offsets into the 4x-tiled local-bias tile
TOT = 960

# per-gate source of the Z term in layers 2..4: 'id' accumulates Z into
# PSUM via an identity matmul (PE), 'dve' adds Z to PSUM on the DVE.
GATE_MODES = ("id", "id", "id", "id")

LAST_RESULT = None       # BassKernelResults of the most recent run (for test.py)


def _build_program(gate_modes=GATE_MODES):
    nc = bacc.Bacc("TRN2", target_bir_lowering=False, debug=False)

    xt_d = nc.dram_tensor("xt", [K, MC], BF16, kind="ExternalInput")
    wtop_d = nc.dram_tensor("wtop", [K, GF], BF16, kind="ExternalInput")
    w80_d = nc.dram_tensor("w80", [128, 2, GF], FP8, kind="ExternalInput")
    w81_d = nc.dram_tensor("w81", [128, 2, GF], FP8, kind="ExternalInput")
    wl_d = nc.dram_tensor("wl", [U, TOT], BF16, kind="ExternalInput")
    wg_d = nc.dram_tensor("wg", [K + U, TOT], BF16, kind="ExternalInput")
    ident_d = nc.dram_tensor("ident", [128, 128], BF16, kind="ExternalInput")
    bgate_d = nc.dram_tensor("bgate", [128, 16], F32, kind="ExternalInput")
    bl4_d = nc.dram_tensor("bl4", [128, 3840], BF16, kind="ExternalInput")
    bgrep_d = nc.dram_tensor("bgrep", [128, TOT], BF16, kind="ExternalInput")
    # bf16 outputs: f32 DVE writes run at half rate and double the DMA bytes
    oloc_d = nc.dram_tensor("oloc", [MC, TOT], BF16, kind="ExternalOutput")
    oglb_d = nc.dram_tensor("oglb", [MC, TOT], BF16, kind="ExternalOutput")

    with tile.TileContext(nc) as tc:
        with (
            tc.tile_pool(name="wpool", bufs=1) as wpool,
            tc.tile_pool(name="xpool", bufs=4) as xpool,
            tc.tile_pool(name="zpool", bufs=2) as zpool,
            tc.tile_pool(name="gpool", bufs=2) as gpool,
            tc.tile_pool(name="cpool", bufs=2) as cpool,
            tc.tile_pool(name="hpool", bufs=4) as hpool,
            tc.tile_pool(name="h8pool", bufs=3) as h8pool,
            tc.tile_pool(name="ttp", bufs=2) as ttp,
            tc.tile_pool(name="tcp", bufs=2) as tcp,
            tc.tile_pool(name="prep", bufs=2) as prep,
            tc.tile_pool(name="lop", bufs=2) as lop,
            tc.tile_pool(name="glop", bufs=3) as glop,
            tc.tile_pool(name="gpsum", bufs=4, space="PSUM") as gpsum,
        ):
            # ---- resident weights/biases --------------------------------
            # First Z matmul needs only x(q0) + the g=0 column group of
            # W_top, so those bytes are DMAed first.
            wtop_sb = [[None] * 4 for _ in range(4)]   # [kt][g]
            xs = {}

            def dma_wtop(g):
                for kt in range(4):
                    t = wpool.tile([128, 512], BF16, tag=f"wt{kt}g{g}")
                    nc.sync.dma_start(
                        t[:], wtop_d[kt * 128:(kt + 1) * 128,
                                     g * 512:(g + 1) * 512])
                    wtop_sb[kt][g] = t

            def dma_x(q):
                xs[q] = []
                for kt in range(4):
                    t = xpool.tile([128, EXQ], BF16, tag=f"x{kt}")
                    nc.sync.dma_start(
                        t[:], xt_d[kt * 128:(kt + 1) * 128,
                                   q * EXQ:(q + 1) * EXQ])
                    xs[q].append(t)

            dma_wtop(0)
            dma_x(0)
            dma_x(1)
            bgate_sb = wpool.tile([128, 16], F32, tag="bgate")
            nc.sync.dma_start(bgate_sb[:], bgate_d[:])
            dma_wtop(1)
            dma_wtop(2)
            dma_wtop(3)
            w8_sb = []
            for j, d in enumerate((w80_d, w81_d)):
                t = wpool.tile([128, 2, GF], FP8, tag=f"w8{j}")
                nc.sync.dma_start(t[:], d[:])
                w8_sb.append(t)
            id_sb = wpool.tile([128, 128], BF16, tag="ident")
            nc.sync.dma_start(id_sb[:], ident_d[:])
            wl_sb = []
            for kt in range(4):
                t = wpool.tile([128, TOT], BF16, tag=f"wl{kt}")
                nc.sync.dma_start(t[:], wl_d[kt * 128:(kt + 1) * 128, :])
                wl_sb.append(t)
            bl4_sb = wpool.tile([128, 3840], BF16, tag="bl4")
            nc.sync.dma_start(bl4_sb[:], bl4_d[:])
            wg_sb = []
            for kt in range(8):
                t = wpool.tile([128, TOT], BF16, tag=f"wg{kt}")
                nc.sync.dma_start(t[:], wg_d[kt * 128:(kt + 1) * 128, :])
                wg_sb.append(t)
            bgrep_sb = wpool.tile([128, TOT], BF16, tag="bgrep")
            nc.sync.dma_start(bgrep_sb[:], bgrep_d[:])

            zs = {}      # (q) -> [4 Z tiles, [128, 2048] bf16, gate-major]
            cs = {}      # (q) -> c tile [128, 2048] bf16
            hs = {}      # (q, layer) -> h tile [128, 2048] bf16
            h8s = {}     # (q, layer) -> h8 tile [128, 4, 512] fp8

            def z_phase(q):
                """Z_g = x @ Wtop_g + b_g for the 4 gates (bf16, in SBUF)."""
                if q in zs:
                    return
                if q not in xs:
                    dma_x(q)
                zs[q] = []
                for g in range(4):
                    zt = zpool.tile([128, GF], BF16, tag=f"z{g}")
                    for half in range(2):
                        ps = gpsum.tile([128, 1024], F32, tag="ps",
                                        name="ps")
                        for ti in range(2):
                            t = half * 2 + ti
                            sl = slice(ti * 512, (ti + 1) * 512)
                            for kt in range(4):
                                nc.tensor.matmul(
                                    ps[:, sl],
                                    wtop_sb[kt][g][:, t * 128:(t + 1) * 128],
                                    xs[q][kt][:],
                                    start=(kt == 0), stop=(kt == 3))
                        for ti in range(2):
                            t = half * 2 + ti
                            of = g * 4 + t
                            sl = slice(ti * 512, (ti + 1) * 512)
                            zsl = slice(t * 512, (t + 1) * 512)
                            # GPSIMD cannot read PSUM -> stays on the DVE
                            nc.vector.tensor_scalar(
                                zt[:, zsl], ps[:, sl],
                                bgate_sb[:, of:of + 1], None, ALU.add)
                    zs[q].append(zt)

            def cand(q, layer, G):
                """c = G_i*G_c (+ G_f*c); h = G_o * tanh(c); h8 = fp8(h)."""
                ht = hpool.tile([128, GF], BF16, tag="h")
                h8t = None
                if layer < 4:   # layer 4's h feeds no further recurrence
                    h8t = h8pool.tile([128, 4, 512], FP8, tag="h8", name="h8t")
                for j in range(2):
                    sl = slice(j * 1024, (j + 1) * 1024)
                    if layer == 1:
                        nc.vector.tensor_mul(
                            cs[q][:, sl], G[1][:, sl], G[3][:, sl])
                    else:
                        t1 = ttp.tile([128, 1024], BF16, tag="t1")
                        nc.vector.tensor_mul(t1[:], G[1][:, sl], G[3][:, sl])
                        t2 = ttp.tile([128, 1024], BF16, tag="t2")
                        # f-gate is the first activation done -> this mul is
                        # off the critical path and GPSIMD is otherwise idle
                        nc.gpsimd.tensor_mul(t2[:], G[0][:, sl], cs[q][:, sl])
                        nc.vector.tensor_add(cs[q][:, sl], t1[:], t2[:])
                    tc_t = tcp.tile([128, 1024], BF16, tag="tc")
                    nc.scalar.activation(tc_t[:], cs[q][:, sl], AF.Tanh)
                    nc.vector.tensor_mul(ht[:, sl], G[2][:, sl], tc_t[:])
                    if h8t is not None:
                        # DVE cast (~0.7us) beats the GPSIMD CAST (~3.6us)
                        # which sat on the h8 -> next-layer-matmul chain
                        nc.vector.tensor_copy(h8t[:, 2 * j:2 * j + 2, :],
                                              ht[:, sl])
                hs[(q, layer)] = ht
                if h8t is not None:
                    h8s[(q, layer)] = h8t

            def l1(q):
                """Layer 1: h0 = 0, so gates come straight from Z."""
                cs[q] = cpool.tile([128, GF], BF16, tag="c", name="c")
                G = [None] * 4
                for g in (1, 2, 3):
                    gt = gpool.tile([128, GF], BF16, tag=f"G{g}")
                    func = AF.Tanh if g == 3 else AF.Sigmoid
                    for jj in range(2):
                        jsl = slice(jj * 1024, (jj + 1) * 1024)
                        nc.scalar.activation(gt[:, jsl], zs[q][g][:, jsl],
                                             func)
                    G[g] = gt
                cand(q, 1, G)

            def rec(q, layer):
                """Layers 2..4: z = Z + h_prev @ W_bot (fp8 DoubleRow)."""
                h8p = h8s[(q, layer - 1)]
                G = []
                for g in range(4):
                    gt = gpool.tile([128, GF], BF16, tag=f"G{g}")
                    func = AF.Tanh if g == 3 else AF.Sigmoid
                    use_id = gate_modes[g] == "id"
                    for half in range(2):
                        ps = gpsum.tile([128, 1024], F32, tag="ps",
                                        name="ps")
                        if use_id:
                            # ids batched before the DR pairs: one dtype /
                            # perf-mode switch per half instead of four
                            for ti in range(2):
                                t = half * 2 + ti
                                sl = slice(ti * 512, (ti + 1) * 512)
                                zsl = slice(t * 512, (t + 1) * 512)
                                nc.tensor.matmul(
                                    ps[:, sl], id_sb[:], zs[q][g][:, zsl],
                                    start=True, stop=False,
                                    skip_group_check=True)
                        for ti in range(2):
                            t = half * 2 + ti
                            sl = slice(ti * 512, (ti + 1) * 512)
                            col = (g * 4 + t) * 128
                            nc.tensor.matmul(
                                ps[:, sl], w8_sb[0][:, :, col:col + 128],
                                h8p[:, 0:2, :],
                                start=(not use_id), stop=False,
                                perf_mode=DR, skip_group_check=True)
                            nc.tensor.matmul(
                                ps[:, sl], w8_sb[1][:, :, col:col + 128],
                                h8p[:, 2:4, :],
                                start=False, stop=True,
                                perf_mode=DR, skip_group_check=True)
                        jsl = slice(half * 1024, (half + 1) * 1024)
                        if use_id:
                            nc.scalar.activation(gt[:, jsl], ps[:], func)
                        else:
                            pre = prep.tile([128, GF], BF16, tag=f"pre{g}")
                            nc.vector.tensor_tensor(
                                pre[:, jsl], ps[:], zs[q][g][:, jsl],
                                ALU.add)
                            nc.scalar.activation(gt[:, jsl], pre[:, jsl],
                                                 func)
                    G.append(gt)
                cand(q, layer, G)

            def loc(q, layer):
                """local_{layer} = h_{layer} @ Wl + bl, natural layout."""
                li = layer - 1
                off, ncl = OFFS[li], NCLS[li]
                ht = hs[(q, layer)]
                st = lop.tile([128, 2048], BF16, tag="lo")
                for half in range(2):
                    ps = gpsum.tile([128, 1024], F32, tag="ps", name="ps")
                    for ei in range(2):
                        e = half * 2 + ei
                        osl = slice(ei * ncl, (ei + 1) * ncl)
                        for t in range(4):
                            nc.tensor.matmul(
                                ps[:, osl],
                                ht[:, t * 512 + e * 128:
                                    t * 512 + e * 128 + 128],
                                wl_sb[t][:, off:off + ncl],
                                start=(t == 0 and (ei * ncl) % 512 == 0),
                                stop=(t == 3 and ei == 1),
                                skip_group_check=True)
                    w2 = 2 * ncl
                    b0 = BL4OFF[li] + half * w2
                    nc.vector.tensor_tensor(
                        st[:, half * w2:half * w2 + w2], ps[:, 0:w2],
                        bl4_sb[:, b0:b0 + w2], ALU.add)
                for e in range(4):
                    r0 = q * EXQ + e * 128
                    nc.sync.dma_start(
                        oloc_d[r0:r0 + 128, off:off + ncl],
                        st[:, e * ncl:(e + 1) * ncl])

            def gl_ep(q, ep):
                """global = [x, h4] @ Wg + bg for one pair of e-tiles."""
                h4 = hs[(q, 4)]
                for ei in range(2):
                    e = ep * 2 + ei
                    ps = gpsum.tile([128, 1024], F32, tag="ps", name="ps")
                    for s0, s1 in ((0, 512), (512, TOT)):
                        for kt in range(8):
                            if kt < 4:
                                lh = xs[q][kt][:, e * 128:(e + 1) * 128]
                            else:
                                t = kt - 4
                                lh = h4[:, t * 512 + e * 128:
                                        t * 512 + e * 128 + 128]
                            nc.tensor.matmul(
                                ps[:, s0:s1], lh, wg_sb[kt][:, s0:s1],
                                start=(kt == 0), stop=(kt == 7),
                                skip_group_check=True)
                    st = glop.tile([128, TOT], BF16, tag="glo", name="glo")
                    nc.vector.tensor_tensor(
                        st[:], ps[:, 0:TOT], bgrep_sb[:], ALU.add)
                    r0 = q * EXQ + e * 128
                    nc.sync.dma_start(oglb_d[r0:r0 + 128, :], st[:])

            # ---- schedule ----------------------------------------------
            for a, b in ((0, 1), (2, 3)):
                z_phase(a)
                z_phase(b)
                l1(a)
                l1(b)
                for layer in (2, 3, 4):
                    rec(a, layer)
                    rec(b, layer)
                    loc(a, layer - 1)
                    loc(b, layer - 1)
                if b == 1:
                    z_phase(2)
                    loc(a, 4)
                    gl_ep(a, 0)
                    gl_ep(a, 1)
                    z_phase(3)
                    loc(b, 4)
                    gl_ep(b, 0)
                    gl_ep(b, 1)
                else:
                    loc(a, 4)
                    gl_ep(a, 0)
                    gl_ep(a, 1)
                    loc(b, 4)
                    gl_ep(b, 0)
                    gl_ep(b, 1)

    nc.compile()
    return nc


_PROGRAM = None


def _get_program():
    global _PROGRAM
    if _PROGRAM is None:
        _PROGRAM = _build_program()
    return _PROGRAM


def _prep_weights(Wf, Wi, Wo, Wc, bf, bi, bo, bc,
                  Wl0, bl0, Wl1, bl1, Wl2, bl2, Wl3, bl3, Wg, bg):
    bf16 = ml_dtypes.bfloat16
    e4m3 = ml_dtypes.float8_e4m3

    wcat = np.concatenate(
        [np.asarray(w, np.float32) for w in (Wf, Wi, Wo, Wc)], axis=1)
    wtop = np.ascontiguousarray(wcat[:K]).astype(bf16)          # [512, 2048]
    wbot = wcat[K:]                                             # [512, 2048]
    wb = wbot.reshape(2, 2, 128, GF)                            # [j, i, p, m]
    w80 = np.ascontiguousarray(wb[0].transpose(1, 0, 2)).astype(e4m3)
    w81 = np.ascontiguousarray(wb[1].transpose(1, 0, 2)).astype(e4m3)

    bcat = np.concatenate(
        [np.asarray(x, np.float32) for x in (bf, bi, bo, bc)])  # [2048]
    bgate = np.ascontiguousarray(bcat.reshape(16, 128).T)       # [128, 16]

    wl = np.concatenate(
        [np.asarray(w, np.float32) for w in (Wl0, Wl1, Wl2, Wl3)],
        axis=1).astype(bf16)                                    # [512, 960]
    blcat = np.concatenate(
        [np.asarray(x, np.float32) for x in (bl0, bl1, bl2, bl3)])
    bl4 = np.concatenate(
        [np.tile(blcat[OFFS[i]:OFFS[i] + NCLS[i]], 4) for i in range(4)])
    bl4 = np.ascontiguousarray(
        np.broadcast_to(bl4, (128, 3840))).astype(bf16)
    wg = np.asarray(Wg, np.float32).astype(bf16)                # [1024, 960]
    bgrep = np.ascontiguousarray(
        np.broadcast_to(np.asarray(bg, np.float32), (128, TOT))).astype(bf16)
    ident = np.eye(128, dtype=np.float32).astype(bf16)

    return {
        "wtop": wtop, "w80": w80, "w81": w81, "wl": wl, "wg": wg,
        "ident": ident, "bgate": bgate, "bl4": bl4, "bgrep": bgrep,
    }


def kernel(inputs, Wf, bf, Wi, bi, Wo, bo, Wc, bc,
           Wl0, bl0, Wl1, bl1, Wl2, bl2, Wl3, bl3, Wg, bg):
    global LAST_RESULT
    bf16 = ml_dtypes.bfloat16

    inputs = np.ascontiguousarray(np.asarray(inputs, dtype=np.float32))
    xt_all = inputs.T.astype(bf16)                    # [512, 16384]
    shared = _prep_weights(Wf, Wi, Wo, Wc, bf, bi, bo, bc,
                           Wl0, bl0, Wl1, bl1, Wl2, bl2, Wl3, bl3, Wg, bg)

    in_maps = []
    for c in range(N_CORES):
        m = {"xt": np.ascontiguousarray(xt_all[:, c * MC:(c + 1) * MC])}
        m.update(shared)
        in_maps.append(m)

    nc = _get_program()
    trace = os.environ.get("BASS_KERNEL_TRACE", "0") == "1"
    tmpdir = os.environ.get("BASS_KERNEL_TMPDIR") or None
    res = run_bass_kernel_spmd(
        nc, in_maps, list(range(N_CORES)), trace=trace, tmpdir=tmpdir)
    LAST_RESULT = res

    loc = np.concatenate(
        [np.asarray(r["oloc"], np.float32) for r in res.results], axis=0)
    glb = np.concatenate(
        [np.asarray(r["oglb"], np.float32) for r in res.results], axis=0)
    return loc, glb
